# revision 1
# baseline (speedup 1.0000x reference)
"""Trainium2 Bass kernel for nn_CausalCrossConditionalSelfAttention.

Strategy (8 NeuronCores, data-parallel over batch B=8, one element/core):
  - Host permutes tokens to interleaved temporal order => causal mask becomes
    lower-triangular, local mask becomes a narrow band (+2 prefix cols).
  - On-chip: qT/kT computed transposed [head*64, T]; v computed [T, head*65]
    with a ones column appended per head so each attention*V matmul also
    produces the softmax denominator Z as output row 64 (free Z reduction).
  - Scores computed transposed S^T[j, i] (keys on partitions) in 128x384
    blocks; fully-masked blocks skipped, fully-unmasked blocks unmasked,
    partial blocks multiplied by host-precomputed 0/1 mask tiles post-exp.
  - Conditional CLIP-token bias folded into the exp() activation bias column.
  - softmax normalization deferred: y~ = P_unnorm @ [v|1]; y = y~[:64] * (1/Z)
    broadcast via gpsimd partition_broadcast; mix-head combination folded in.
  - b_value/b_proj folded into a constant host-side output shift.
  - Matmuls run as float32r (full PE rate at N>=256, ~fp32 precision).

Self-contained: only needs numpy + the installed concourse/bass stack.
"""

import sys

if "/opt/trn_rl_repo" not in sys.path:
    sys.path.insert(0, "/opt/trn_rl_repo")

import numpy as np

# ----------------------------------------------------------------------------
# problem constants (hardcoded per spec)
# ----------------------------------------------------------------------------
BLOCK = 512
RECEP = 4
N_HEAD = 8
EMBED = 512
HS = 64
T = 2 * BLOCK + 2          # 1026
TP = 1152                  # 9 * 128
W = 384                    # query-chunk width, 3 chunks
NIC = TP // W
NJB = TP // 128
NSM = 10                   # softmaxes: heads 0..7, ml0 (s=8), ml1 (s=9)
NCORES = 8

# softmax id -> (mask kind, q/k source, v head)
SM_INFO = [
    (0, "loc", "main", 0), (1, "loc", "main", 1),
    (2, "seq", "main", 2), (3, "seq", "main", 3),
    (4, "seq", "main", 4), (5, "seq", "main", 5),
    (6, "seq", "main", 6), (7, "seq", "main", 7),
    (8, "loc", "ml", 2), (9, "loc", "ml", 3),
]
# emission order: heavy causal heads first, band heads last
SM_ORDER = [2, 0, 3, 1, 4, 8, 5, 9, 6, 7]


# ----------------------------------------------------------------------------
# host-side plan construction
# ----------------------------------------------------------------------------
def build_perm():
    perm = np.zeros(T, dtype=np.int64)
    perm[0], perm[1] = 0, 1
    b = np.arange(BLOCK)
    perm[2 + 2 * b] = 2 + b
    perm[3 + 2 * b] = 2 + BLOCK + b
    inv = np.argsort(perm)
    return perm, inv


def build_masks_orig():
    to = np.concatenate([np.zeros(2), np.arange(BLOCK) * 2 + 1, np.arange(BLOCK) * 2 + 2])
    seq = to[None, :] <= to[:, None]
    qo = np.concatenate([np.arange(BLOCK) * 2 + 1 - 2 * RECEP + 1] * 2)
    ko = np.concatenate([np.arange(BLOCK) * 2 + 1] * 2)
    de = ko[None, :] < qo[:, None]
    loc = seq.copy()
    loc[2:, 2:] = loc[2:, 2:] & (~de)
    return seq, loc


def build_block_plan():
    perm, _ = build_perm()
    seq, loc = build_masks_orig()
    Ms = np.zeros((TP, TP), dtype=bool)
    Ml = np.zeros((TP, TP), dtype=bool)
    Ms[:T, :T] = seq[perm][:, perm]
    Ml[:T, :T] = loc[perm][:, perm]
    # padded query rows are don't-care: replicate last real query row so
    # blocks classify as 'full'; padded key columns stay masked.
    Ms[T:] = Ms[T - 1]
    Ml[T:] = Ml[T - 1]

    mask_tiles = []
    tile_index = {}

    def tile_id(tile):
        key = tile.tobytes()
        if key not in tile_index:
            tile_index[key] = len(mask_tiles)
            mask_tiles.append(tile)
        return tile_index[key]

    plans = {}
    for kind, M in (("seq", Ms), ("loc", Ml)):
        plan = []
        for ic in range(NIC):
            blocks = []
            for jb in range(NJB):
                sub = M[ic * W:(ic + 1) * W, jb * 128:(jb + 1) * 128].T  # [128, W]
                if not sub.any():
                    continue
                nz_rows = np.flatnonzero(sub.any(axis=1))
                if sub.all():
                    blocks.append((jb, "full", None))
                elif jb == 0 and nz_rows.max() <= 1 and sub[nz_rows].all():
                    blocks.append((jb, "prefix", int(nz_rows.max()) + 1))
                else:
                    zcols = np.flatnonzero(~sub.all(axis=0))
                    c0, c1 = int(zcols.min()), int(zcols.max()) + 1
                    mid = tile_id(sub[:, c0:c1].astype(np.float32).copy())
                    blocks.append((jb, "part", (mid, c0, c1)))
            plan.append(blocks)
        plans[kind] = plan
    # concatenate cropped masks along the free dim; record offsets
    offs, cat = [], []
    o = 0
    for t in mask_tiles:
        offs.append((o, t.shape[1]))
        cat.append(t)
        o += t.shape[1]
    maskcat = np.concatenate(cat, axis=1) if cat else np.zeros((128, 0), np.float32)
    return plans, (maskcat, offs)


def prep_core_inputs(x_b, cond_b, w):
    """Per-core input tensors (numpy fp32) for the bass kernel."""
    perm, _ = build_perm()
    scale = np.float32(1.0 / np.sqrt(HS))

    xT = np.zeros((EMBED, TP), dtype=np.float32)
    xT[:, :T] = x_b[perm].T

    f = np.float32
    wq = np.ascontiguousarray(w["w_query"].astype(f).T * scale)
    wk = np.ascontiguousarray(w["w_key"].astype(f).T)
    wv = np.ascontiguousarray(w["w_value"].astype(f).T)
    wp = np.ascontiguousarray(w["w_proj"].astype(f).T)
    wqml = np.ascontiguousarray(w["w_query_ml"].astype(f).T * scale)
    wkml = np.ascontiguousarray(w["w_key_ml"].astype(f).T)

    bq = np.ascontiguousarray((w["b_query"].astype(f) * scale).reshape(4, 128).T)
    bk = np.ascontiguousarray(w["b_key"].astype(f).reshape(4, 128).T)
    bqml = (w["b_query_ml"].astype(f) * scale).reshape(128, 1).copy()
    bkml = w["b_key_ml"].astype(f).reshape(128, 1).copy()

    clip8 = np.maximum(w["att_bias_clip"].astype(f)[0, :, 0], 0.0) * 10.0
    clip2 = np.maximum(w["att_bias_clip_ml"].astype(f)[0, :, 0], 0.0) * 10.0
    biascols = np.zeros((128, NSM), dtype=f)
    if cond_b > 0:
        biascols[1, :N_HEAD] = clip8
        biascols[1, N_HEAD:] = clip2

    wg = w["w_mix"].astype(f)[:, 0, 0, 0]
    wl = w["w_mix"].astype(f)[:, 1, 0, 0]
    mixcol_s = np.ones(NSM, dtype=f)
    mixcol_s[2], mixcol_s[3] = wg[0], wg[1]
    mixcol_s[8], mixcol_s[9] = wl[0], wl[1]
    mixcol = mixcol_s[np.array(SM_ORDER)].reshape(NSM, 1).copy()

    return dict(xT=xT, wq=wq, wk=wk, wv=wv, wp=wp, wqml=wqml, wkml=wkml,
                bq=bq, bk=bk, bqml=bqml, bkml=bkml,
                biascols=biascols, mixcol=mixcol,
                ones8=np.ones((128, N_HEAD), dtype=f),
                onesrow=np.ones((1, 128), dtype=f))


def host_const_shift(w):
    bv = w["b_value"].astype(np.float64)
    wg = w["w_mix"].astype(np.float64)[:, 0, 0, 0]
    wl = w["w_mix"].astype(np.float64)[:, 1, 0, 0]
    scale_h = np.ones(N_HEAD)
    scale_h[2] = wg[0] + wl[0]
    scale_h[3] = wg[1] + wl[1]
    yshift = (bv.reshape(N_HEAD, HS) * scale_h[:, None]).reshape(-1)
    return (yshift @ w["w_proj"].astype(np.float64).T
            + w["b_proj"].astype(np.float64)).astype(np.float32)


# ----------------------------------------------------------------------------
# bass kernel emission
# ----------------------------------------------------------------------------
def emit_kernel(tc, ins, out_ap, plans, n_masks):
    from contextlib import ExitStack
    from concourse import mybir

    nc = tc.nc
    f32 = mybir.dt.float32
    f32r = mybir.dt.float32r
    AF = mybir.ActivationFunctionType

    def r(ap):
        return ap.bitcast(f32r)

    with ExitStack() as ctx:
        P = ctx.enter_context(tc.tile_pool(name="persist", bufs=1))
        xpool = ctx.enter_context(tc.tile_pool(name="xp", bufs=1))
        xT = [xpool.tile([128, TP], f32, name=f"x{k}", tag=f"x{k}") for k in range(4)]

        def loadw(name, ap, kchunks, ncols, eng=None):
            eng = eng or nc.sync
            tiles = []
            for kc in range(kchunks):
                t = P.tile([128, ncols], f32, name=f"{name}{kc}", tag=f"{name}{kc}")
                eng.dma_start(r(t[:]), r(ap[kc * 128:(kc + 1) * 128, :]))
                tiles.append(t)
            return tiles

        def loadw1(name, ap, kc, ncols, eng):
            t = P.tile([128, ncols], f32, name=f"{name}{kc}", tag=f"{name}{kc}")
            eng.dma_start(r(t[:]), r(ap[kc * 128:(kc + 1) * 128, :]))
            return t

        # interleave x chunks with the weight chunks each projection matmul
        # needs first, split across the two HWDGE queues
        nc.sync.dma_start(r(xT[0][:]), r(ins["xT"][0:128, :]))
        nc.scalar.dma_start(r(xT[2][:]), r(ins["xT"][256:384, :]))
        wq_sb = [loadw1("wq", ins["wq"], 0, 512, nc.sync)]
        wk_sb = [loadw1("wk", ins["wk"], 0, 512, nc.scalar)]
        nc.sync.dma_start(r(xT[1][:]), r(ins["xT"][128:256, :]))
        nc.scalar.dma_start(r(xT[3][:]), r(ins["xT"][384:512, :]))
        for kc in range(1, 4):
            wq_sb.append(loadw1("wq", ins["wq"], kc, 512, nc.sync))
            wk_sb.append(loadw1("wk", ins["wk"], kc, 512, nc.scalar))
        wv_sb = loadw("wv", ins["wv"], 4, 512)
        wqml_sb = loadw("wqml", ins["wqml"], 4, 128, nc.scalar)
        wkml_sb = loadw("wkml", ins["wkml"], 4, 128, nc.scalar)

        def loads(name, shape):
            t = P.tile(list(shape), f32, name=name, tag=name)
            nc.sync.dma_start(t[:], ins[name][:, :])
            return t

        bq_sb = loads("bq", (128, 4))
        bk_sb = loads("bk", (128, 4))
        bqml_sb = loads("bqml", (128, 1))
        bkml_sb = loads("bkml", (128, 1))
        biascols_sb = loads("biascols", (128, NSM))
        mixcol_sb = loads("mixcol", (NSM, 1))
        ins_onesrow = P.tile([1, 128], f32, name="onesrow", tag="onesrow")
        nc.sync.dma_start(r(ins_onesrow[:]), r(ins["onesrow"][:, :]))



        # persistent compute tiles
        qT = [P.tile([128, TP], f32, name=f"qT{m}", tag=f"qT{m}") for m in range(4)]
        kT = [P.tile([128, TP], f32, name=f"kT{m}", tag=f"kT{m}") for m in range(4)]
        qml = P.tile([128, TP], f32, name="qml", tag="qml")
        kml = P.tile([128, TP], f32, name="kml", tag="kml")
        vext = [P.tile([128, N_HEAD * 65], f32, name=f"vext{t}", tag=f"vext{t}")
                for t in range(NJB)]
        yTn = [P.tile([128, TP], f32, name=f"yTn{p}", tag=f"yTn{p}") for p in range(4)]
        tmpml = P.tile([128, TP], f32, name="tmpml", tag="tmpml")
        zall = P.tile([NSM, TP], f32, name="zall", tag="zall")
        rall = P.tile([NSM, TP], f32, name="rall", tag="rall")
        zorder = {s: i for i, s in enumerate(SM_ORDER)}

        if globals().get("DEBUG_TILES"):
            global _LAST_TILES
            _LAST_TILES = dict(qT=qT, kT=kT, qml=qml, kml=kml, vext=vext,
                               yTn=yTn, tmpml=tmpml, zall=zall)

        ptp = ctx.enter_context(tc.tile_pool(name="ptp", bufs=6))
        ostage = ctx.enter_context(tc.tile_pool(name="ostage", bufs=2))
        ztp = ctx.enter_context(tc.tile_pool(name="ztp", bufs=2))

        # ---------------- phase 1: projections ----------------
        with tc.tile_pool(name="pps", bufs=2, space="PSUM") as pps, \
             tc.tile_pool(name="vps", bufs=2, space="PSUM") as vps:

            def proj_T(wtiles, bias, dst_tiles, mchunks):
                # dst[c_out, t] = sum_c w[c, c_out] x[c, t] (+ bias[c_out]);
                # all 3 query chunks accumulate into one 3-bank psum group so
                # the evacuation is a single wide ACT op.
                for m in range(mchunks):
                    dst = dst_tiles[m] if mchunks > 1 else dst_tiles[0]
                    ps = pps.tile([128, 1536], f32, name="pp", tag="pp")
                    for ic in range(NIC):
                        for kc in range(4):
                            nc.tensor.matmul(
                                ps[:, ic * 512:ic * 512 + W],
                                lhsT=r(wtiles[kc][:, m * 128:(m + 1) * 128]),
                                rhs=r(xT[kc][:, ic * W:(ic + 1) * W]),
                                start=(kc == 0), stop=(kc == 3))
                    nc.vector.tensor_scalar_add(
                        r(dst[:].rearrange("p (g w) -> p g w", w=W)),
                        ps[:].rearrange("p (g c) -> p g c", c=512)[:, :, 0:W],
                        bias[:, m:m + 1])

            proj_T(wq_sb, bq_sb, qT, 4)
            proj_T(wk_sb, bk_sb, kT, 4)
            proj_T(wqml_sb, bqml_sb, [qml], 1)
            proj_T(wkml_sb, bkml_sb, [kml], 1)

            # v~ [t, 8*65] with ones col per head (ones via DMA)
            for tt in range(NJB):
                ps = vps.tile([128, 512], f32, name="vp", tag="vp")
                for kc in range(4):
                    nc.tensor.matmul(
                        ps[:],
                        lhsT=r(xT[kc][:, tt * 128:(tt + 1) * 128]),
                        rhs=r(wv_sb[kc][:]),
                        start=(kc == 0), stop=(kc == 3))
                vx = vext[tt][:].rearrange("p (h e) -> p h e", e=65)
                nc.scalar.activation(
                    r(vx[:, :, 0:64]), ps[:].rearrange("p (h d) -> p h d", d=64),
                    AF.Copy)
                nc.sync.dma_start(r(vx[:, :, 64:65]),
                                  r(ins["ones8"][:, :, None]))

        # late loads: wp (phase 4) and masks (phase 2 partial blocks)
        mask_offs = n_masks[1]
        maskw = n_masks[0]
        maskcat_sb = P.tile([128, maskw], f32, name="maskcat", tag="maskcat")
        nc.scalar.dma_start(maskcat_sb[:], ins["masks"][:, :])
        wp_sb = loadw("wp", ins["wp"], 4, 512)

        # ---------------- phase 2: attention softmaxes ----------------
        with tc.tile_pool(name="spsum", bufs=5, space="PSUM") as spsum, \
             tc.tile_pool(name="jps", bufs=1, space="PSUM") as jps, \
             tc.tile_pool(name="ypsum", bufs=2, space="PSUM") as ypsum:
            for s in SM_ORDER:
                _, kindname, src_, hv = SM_INFO[s]
                if src_ == "main":
                    qt, kt, off = qT[s // 2], kT[s // 2], (s % 2) * 64
                else:
                    qt, kt, off = qml, kml, (s - N_HEAD) * 64
                plan = plans[kindname]
                for ic in range(NIC):
                    i0 = ic * W
                    blocks = plan[ic]
                    Y = ypsum.tile([128, 512], f32, name="y", tag="y")
                    n_av = len(blocks)
                    avi = 0

                    def av(pt_ap, jb, rows=128):
                        nonlocal avi
                        nc.tensor.matmul(
                            Y[0:65, :W],
                            lhsT=r(vext[jb][0:rows, hv * 65:hv * 65 + 65]),
                            rhs=r(pt_ap),
                            start=(avi == 0), stop=(avi == n_av - 1))
                        avi += 1

                    units = [("one", [blk]) if (blk[0] == 0 or blk[1] == "prefix")
                             else ("pair", [blk]) for blk in blocks]

                    for kind_u, blks in units:
                        if kind_u == "one":
                            jb, bt, aux = blks[0]
                            ps = jps.tile([128, 512], f32, name="jp", tag="jp")
                            rows = aux if bt == "prefix" else 128
                            nc.tensor.matmul(
                                ps[0:rows, :W],
                                lhsT=r(kt[off:off + 64, 0:rows]) if bt == "prefix"
                                else r(kt[off:off + 64, 0:128]),
                                rhs=r(qt[off:off + 64, i0:i0 + W]),
                                start=True, stop=True)
                            pt = ptp.tile([128, W], f32, name="pt0", tag="pt0", bufs=3)
                            nc.scalar.activation(
                                r(pt[0:rows, :]), ps[0:rows, :W], AF.Exp,
                                bias=biascols_sb[0:rows, s:s + 1], scale=1.0)
                            if bt == "part":
                                mid, c0, c1 = aux
                                mo, mw = mask_offs[mid]
                                eng = nc.vector if kindname == "seq" else nc.gpsimd
                                eng.tensor_mul(r(pt[:, c0:c1]), pt[:, c0:c1],
                                               maskcat_sb[:, mo:mo + mw])
                            av(pt[0:rows, :], jb, rows)
                        else:
                            ps = spsum.tile([128, 512], f32, name="sp", tag="sp")
                            for g, (jb, bt, aux) in enumerate(blks):
                                nc.tensor.matmul(
                                    ps[:, g * 512:g * 512 + W],
                                    lhsT=r(kt[off:off + 64, jb * 128:jb * 128 + 128]),
                                    rhs=r(qt[off:off + 64, i0:i0 + W]),
                                    start=True, stop=True)
                            ng = len(blks)
                            pt = ptp.tile([128, ng * W], f32, name="pt", tag="pt")
                            nc.scalar.activation(
                                r(pt[:].rearrange("p (g w) -> p g w", w=W)),
                                ps[:].rearrange("p (g c) -> p g c", c=512)[:, 0:ng, 0:W],
                                AF.Exp)
                            for g, (jb, bt, aux) in enumerate(blks):
                                if bt == "part":
                                    mid, c0, c1 = aux
                                    mo, mw = mask_offs[mid]
                                    eng = nc.vector if kindname == "seq" else nc.gpsimd
                                    eng.tensor_mul(
                                        r(pt[:, g * W + c0:g * W + c1]),
                                        pt[:, g * W + c0:g * W + c1],
                                        maskcat_sb[:, mo:mo + mw])
                            for g, (jb, bt, aux) in enumerate(blks):
                                av(pt[:, g * W:(g + 1) * W], jb)

                    # evacuate unnormalized y (DVE) and Z row (DVE->DMA)
                    if s < N_HEAD:
                        dst = yTn[s // 2][(s % 2) * 64:(s % 2) * 64 + 64, i0:i0 + W]
                    else:
                        dst = tmpml[(s - N_HEAD) * 64:(s - N_HEAD) * 64 + 64, i0:i0 + W]
                    nc.vector.tensor_copy(r(dst), Y[0:64, :W])
                    zt = ztp.tile([1, W], f32, name="zt", tag="zt", bufs=2)
                    nc.vector.tensor_copy(zt[:], Y[64:65, :W])
                    zrow = zorder[s]
                    nc.sync.dma_start(zall[zrow:zrow + 1, i0:i0 + W], zt[:])
                # progressive reciprocal: rows [0:k] are final once the k-th
                # softmax in SM_ORDER is done (recomputing earlier rows is
                # idempotent - recip reads zall, writes rall)
                if SM_ORDER.index(s) in (3, 6):
                    k = SM_ORDER.index(s) + 1
                    nc.vector.reciprocal(rall[0:k, :], zall[0:k, :])
                    nc.vector.tensor_scalar_mul(rall[0:k, :], rall[0:k, :],
                                                mixcol_sb[0:k, :])

        # ---------------- phase 3+4: normalization + output projection ----
        nc.vector.reciprocal(rall[:], zall[:])
        nc.vector.tensor_scalar_mul(rall[:], rall[:], mixcol_sb[:])

        if globals().get("DEBUG_PRENORM") is not None:
            for _m in range(4):
                nc.sync.dma_start(DEBUG_PRENORM[_m], yTn[_m][:])
            nc.sync.dma_start(DEBUG_PRENORM[4], tmpml[:])

        with tc.tile_pool(name="rbps", bufs=2, space="PSUM") as rbps, \
             tc.tile_pool(name="opsum", bufs=2, space="PSUM") as opsum:
            _rbi = [0]

            def bcast_row(s_idx, eng):
                # recip row s -> partition-0 staging -> PE ones-matmul
                # broadcast into all 128 partitions of a 3-bank psum tile
                zr = ztp.tile([1, TP], f32, name="zr", tag="zr", bufs=2)
                row = zorder[s_idx]
                eng.dma_start(r(zr[:]), r(rall[row:row + 1, :]))
                rb = rbps.tile([128, 1536], f32, name="rb", tag="rb")
                for g in range(NIC):
                    nc.tensor.matmul(
                        rb[:, g * 512:g * 512 + W],
                        lhsT=r(ins_onesrow[0:1, :]),
                        rhs=r(zr[0:1, g * W:(g + 1) * W]),
                        start=True, stop=True)
                return rb

            def norm_pair(dsttile, sa, sb):
                rba = bcast_row(sa, nc.sync)
                rbb = bcast_row(sb, nc.scalar)
                for half, rb in ((0, rba), (1, rbb)):
                    p0 = half * 64
                    nc.vector.tensor_mul(
                        r(dsttile[p0:p0 + 64].rearrange("p (g w) -> p g w", w=W)),
                        dsttile[p0:p0 + 64].rearrange("p (g w) -> p g w", w=W),
                        rb[:].rearrange("p (g c) -> p g c", c=512)[p0:p0 + 64, :, 0:W])

            norm_pair(yTn[0], 0, 1)
            norm_pair(yTn[2], 4, 5)
            norm_pair(yTn[3], 6, 7)
            norm_pair(yTn[1], 2, 3)
            norm_pair(tmpml, 8, 9)
            nc.vector.tensor_add(r(yTn[1][:]), yTn[1][:], tmpml[:])
            for m in range(NJB):
                po = opsum.tile([128, 512], f32, name="po", tag="po")
                for p in range(4):
                    nc.tensor.matmul(
                        po[:],
                        lhsT=r(yTn[p][:, m * 128:(m + 1) * 128]),
                        rhs=r(wp_sb[p][:]),
                        start=(p == 0), stop=(p == 3))
                ost = ostage.tile([128, 512], f32, name="ost", tag="ost")
                nc.vector.tensor_copy(ost[:], po[:])
                eng = nc.sync if m % 2 == 0 else nc.scalar
                eng.dma_start(out_ap[m * 128:(m + 1) * 128, :], ost[:])


# ----------------------------------------------------------------------------
# module build + run
# ----------------------------------------------------------------------------
_CACHE = {}


def _get_module():
    if "nc" in _CACHE:
        return _CACHE["nc"], _CACHE["plans"], _CACHE["mask_tiles"]
    import concourse.tile as tile
    from concourse import bacc, mybir

    plans, (maskcat, mask_offs) = build_block_plan()

    nc = bacc.Bacc("TRN2", target_bir_lowering=False, debug=False,
                   enable_asserts=False, num_devices=NCORES)
    f32 = mybir.dt.float32

    def din(name, shape):
        return nc.dram_tensor(name, list(shape), f32, kind="ExternalInput").ap()

    ins = dict(
        xT=din("xT", (EMBED, TP)),
        wq=din("wq", (EMBED, EMBED)), wk=din("wk", (EMBED, EMBED)),
        wv=din("wv", (EMBED, EMBED)), wp=din("wp", (EMBED, EMBED)),
        wqml=din("wqml", (EMBED, 128)), wkml=din("wkml", (EMBED, 128)),
        bq=din("bq", (128, 4)), bk=din("bk", (128, 4)),
        bqml=din("bqml", (128, 1)), bkml=din("bkml", (128, 1)),
        biascols=din("biascols", (128, NSM)),
        mixcol=din("mixcol", (NSM, 1)),
        masks=din("masks", (128, maskcat.shape[1])),
        ones8=din("ones8", (128, N_HEAD)),
        onesrow=din("onesrow", (1, 128)),
    )
    out_ap = nc.dram_tensor("out_p", [TP, EMBED], f32, kind="ExternalOutput").ap()

    with tile.TileContext(nc) as tc:
        emit_kernel(tc, ins, out_ap, plans, (maskcat.shape[1], mask_offs))
    nc.compile()

    _CACHE.update(nc=nc, plans=plans, mask_tiles=maskcat)
    return nc, plans, maskcat


def build_in_maps(inputs):
    """Per-core input maps; weights/masks prepped once and shared."""
    nc, plans, maskcat = _get_module()
    x = inputs["x"].astype(np.float32)
    cond = np.asarray(inputs["cond_mask"]).astype(np.int32)
    B = x.shape[0]
    assert B == NCORES, f"expected B={NCORES}, got {B}"

    perm, _ = build_perm()
    base0 = prep_core_inputs(x[0], int(cond[0]), inputs)
    base0["masks"] = maskcat
    in_maps = [base0]
    for b in range(1, B):
        ci = dict(base0)
        xT = np.zeros((EMBED, TP), dtype=np.float32)
        xT[:, :T] = x[b][perm].T
        ci["xT"] = xT
        if int(cond[b]) != int(cond[0]):
            biascols = base0["biascols"].copy()
            if int(cond[b]) > 0:
                f = np.float32
                clip8 = np.maximum(inputs["att_bias_clip"].astype(f)[0, :, 0], 0.0) * 10.0
                clip2 = np.maximum(inputs["att_bias_clip_ml"].astype(f)[0, :, 0], 0.0) * 10.0
                biascols[1, :N_HEAD] = clip8
                biascols[1, N_HEAD:] = clip2
            else:
                biascols[:] = 0.0
            ci["biascols"] = biascols
        in_maps.append(ci)
    return nc, in_maps


def kernel(**inputs):
    from concourse import bass_utils

    inputs = {k: np.asarray(v) for k, v in inputs.items()}
    nc, in_maps = build_in_maps(inputs)
    res = bass_utils.run_bass_kernel_spmd(nc, in_maps, core_ids=list(range(NCORES)))
    _CACHE["last_results"] = res

    _, inv = build_perm()
    shift = host_const_shift(inputs)
    B = inputs["x"].shape[0]
    out = np.empty((B, T, EMBED), dtype=np.float32)
    for b in range(B):
        out[b] = res.results[b]["out_p"][:T][inv] + shift
    return out



# revision 5
# speedup vs baseline: 1.1664x; 1.1664x over previous
"""Trainium2 Bass kernel for nn_CausalCrossConditionalSelfAttention.

Strategy (8 NeuronCores, data-parallel over batch B=8, one element/core):
  - Host permutes tokens to interleaved temporal order => causal mask becomes
    lower-triangular, local mask a narrow band (+2 prefix cols).
  - All matmuls in bf16 (1 cyc/row at any N in the TRN2 cost model), psum fp32.
  - Scores computed transposed S^T[key, query] in column-cropped blocks;
    band blocks cropped to their true content span; fully-masked blocks
    skipped; partial blocks multiplied by host-precomputed 0/1 bf16 masks.
  - Conditional CLIP-token bias added via a rank-1 accumulate matmul
    (indicator-row x bias-row) on jb==0 score blocks only.
  - exp() batched: several score blocks share one psum tile and one wide
    Act instruction (Act is the attention-phase bottleneck engine).
  - P@[V|1] gives unnormalized y plus softmax denominator Z as psum row 64;
    both evacuated per (softmax, query-chunk) to an fp32 staging tile; Z
    reciprocal on DVE -> gpsimd partition_broadcast -> fused
    (y * mix) * (1/Z) normalization via scalar_tensor_tensor into yTn bf16.
  - Projections interleaved with attention so PE stays dense (full clock);
    q/k projections for the first softmaxes run first.
  - b_value/b_proj folded into a constant host-side output shift.

Self-contained: only needs numpy + ml_dtypes + the installed concourse stack.
"""

import sys

if "/opt/trn_rl_repo" not in sys.path:
    sys.path.insert(0, "/opt/trn_rl_repo")

import numpy as np
import ml_dtypes

# ----------------------------------------------------------------------------
# problem constants (hardcoded per spec)
# ----------------------------------------------------------------------------
BLOCK = 512
RECEP = 4
N_HEAD = 8
EMBED = 512
HS = 64
T = 2 * BLOCK + 2          # 1026
TP = 1152                  # 9 * 128
NJB = TP // 128
NIC = 3
AW = (384, 384, 258)       # per-ic live query widths (ic2 ends at T=1026)
NSM = 10
NCORES = 8
BF = ml_dtypes.bfloat16

# softmax id -> (mask kind, q/k source, v head)
SM_INFO = [
    (0, "loc", "main", 0), (1, "loc", "main", 1),
    (2, "seq", "main", 2), (3, "seq", "main", 3),
    (4, "seq", "main", 4), (5, "seq", "main", 5),
    (6, "seq", "main", 6), (7, "seq", "main", 7),
    (8, "loc", "ml", 2), (9, "loc", "ml", 3),
]
# emission order: heavy causal heads first, cheap band heads last
SM_ORDER = [2, 3, 4, 5, 6, 7, 0, 8, 1, 9]


# ----------------------------------------------------------------------------
# host-side plan construction
# ----------------------------------------------------------------------------
def build_perm():
    perm = np.zeros(T, dtype=np.int64)
    perm[0], perm[1] = 0, 1
    b = np.arange(BLOCK)
    perm[2 + 2 * b] = 2 + b
    perm[3 + 2 * b] = 2 + BLOCK + b
    inv = np.argsort(perm)
    return perm, inv


def build_masks_orig():
    to = np.concatenate([np.zeros(2), np.arange(BLOCK) * 2 + 1, np.arange(BLOCK) * 2 + 2])
    seq = to[None, :] <= to[:, None]
    qo = np.concatenate([np.arange(BLOCK) * 2 + 1 - 2 * RECEP + 1] * 2)
    ko = np.concatenate([np.arange(BLOCK) * 2 + 1] * 2)
    de = ko[None, :] < qo[:, None]
    loc = seq.copy()
    loc[2:, 2:] = loc[2:, 2:] & (~de)
    return seq, loc


def build_units():
    """Per (kind, ic): list of units.

    unit = dict(rows, w, nfit, blocks=[dict(jb, a0, mid)]); blocks in a unit
    share (rows, w); psum layout: single-bank packed (nfit>1, chunk k at
    col k*w) or one bank per block (nfit==1, chunk k at col k*512).
    Unit 0 contains jb==0 (AV start flag / CLIP bias matmul target).
    """
    perm, _ = build_perm()
    seq, loc = build_masks_orig()
    mask_tiles, tile_index = [], {}

    def tile_id(slab):
        key = (slab.shape[1], slab.tobytes())
        if key not in tile_index:
            tile_index[key] = len(mask_tiles)
            mask_tiles.append(slab.astype(np.float32))
        return tile_index[key]

    plans = {}
    for kind, M0 in (("seq", seq), ("loc", loc)):
        Mp = np.zeros((TP, TP), dtype=bool)
        Mp[:T, :T] = M0[perm][:, perm]
        icunits = []
        for ic in range(NIC):
            i0, W = ic * 384, AW[ic]
            blocks = []
            for jb in range(NJB):
                sub = Mp[i0:i0 + W, jb * 128:(jb + 1) * 128].T.copy()  # [128 keys, W]
                if not sub.any():
                    continue
                nzr = np.flatnonzero(sub.any(axis=1))
                rows = 2 if nzr.max() <= 1 else 128
                nzc = np.flatnonzero(sub.any(axis=0))
                a0 = int(nzc.min()) & ~1
                a1 = min(W, (int(nzc.max()) + 2) & ~1)
                blocks.append((jb, rows, a0, a1, sub))
            # classes by (rows, wide?); unify width within class
            cls = {}
            for b in blocks:
                jb, rows, a0, a1, sub = b
                cls.setdefault((rows, (a1 - a0) >= 256), []).append(b)
            units = []
            for (rows, wide), bl in sorted(cls.items(), key=lambda kv: min(b[0] for b in kv[1])):
                w = W if wide else max(b[3] - b[2] for b in bl)
                nfit = (512 // w) if w < 256 else 1
                cap = nfit if nfit > 1 else 3
                cur = []
                for jb, brows, a0, a1, sub in bl:
                    a0 = max(0, min(a0, W - w)) & ~1
                    slab = sub[:, a0:a0 + w]
                    mid = None if slab[0:rows].all() else tile_id(slab)
                    cur.append(dict(jb=jb, a0=a0, mid=mid))
                    if len(cur) == cap:
                        units.append(dict(rows=rows, w=w, nfit=nfit, blocks=cur))
                        cur = []
                if cur:
                    units.append(dict(rows=rows, w=w, nfit=nfit, blocks=cur))
            units.sort(key=lambda u: min(b["jb"] for b in u["blocks"]))
            assert units[0]["blocks"][0]["jb"] == 0
            icunits.append(units)
        plans[kind] = icunits

    offs, cat, o = [], [], 0
    for t in mask_tiles:
        offs.append((o, t.shape[1]))
        cat.append(t)
        o += t.shape[1]
    maskcat = np.concatenate(cat, axis=1) if cat else np.zeros((128, 2), np.float32)
    return plans, maskcat, offs


def host_const_shift(w):
    bv = w["b_value"].astype(np.float64)
    wg = w["w_mix"].astype(np.float64)[:, 0, 0, 0]
    wl = w["w_mix"].astype(np.float64)[:, 1, 0, 0]
    scale_h = np.ones(N_HEAD)
    scale_h[2] = wg[0] + wl[0]
    scale_h[3] = wg[1] + wl[1]
    yshift = (bv.reshape(N_HEAD, HS) * scale_h[:, None]).reshape(-1)
    return (yshift @ w["w_proj"].astype(np.float64).T
            + w["b_proj"].astype(np.float64)).astype(np.float32)


def prep_shared(w):
    """Shared (weight) tensors, bf16 where they feed matmuls."""
    f = np.float32
    scale = np.float32(1.0 / np.sqrt(HS))
    out = {}
    out["wq"] = np.ascontiguousarray(w["w_query"].astype(f).T * scale).astype(BF)
    out["wk"] = np.ascontiguousarray(w["w_key"].astype(f).T).astype(BF)
    out["wv"] = np.ascontiguousarray(w["w_value"].astype(f).T).astype(BF)
    out["wp"] = np.ascontiguousarray(w["w_proj"].astype(f).T).astype(BF)
    out["wqml"] = np.ascontiguousarray(w["w_query_ml"].astype(f).T * scale).astype(BF)
    out["wkml"] = np.ascontiguousarray(w["w_key_ml"].astype(f).T).astype(BF)
    out["bq"] = np.ascontiguousarray((w["b_query"].astype(f) * scale).reshape(4, 128).T)
    out["bk"] = np.ascontiguousarray(w["b_key"].astype(f).reshape(4, 128).T)
    out["bqml"] = (w["b_query_ml"].astype(f) * scale).reshape(128, 1).copy()
    out["bkml"] = w["b_key_ml"].astype(f).reshape(128, 1).copy()

    wg = w["w_mix"].astype(f)[:, 0, 0, 0]
    wl = w["w_mix"].astype(f)[:, 1, 0, 0]
    mix = np.ones(NSM, dtype=f)
    mix[2], mix[3] = wg[0], wg[1]
    mix[8], mix[9] = wl[0], wl[1]
    out["mixbc"] = np.tile(mix[None, :], (128, 1)).copy()

    ind = np.zeros((1, TP), dtype=f)
    ind[0, 1] = 1.0
    out["indrow"] = ind.astype(BF)
    out["onesh"] = np.ones((128, N_HEAD), dtype=f).astype(BF)
    return out


def prep_biasrow(w, cond_b):
    f = np.float32
    br = np.zeros((1, NSM * TP), dtype=f)
    if cond_b > 0:
        clip8 = np.maximum(w["att_bias_clip"].astype(f)[0, :, 0], 0.0) * 10.0
        clip2 = np.maximum(w["att_bias_clip_ml"].astype(f)[0, :, 0], 0.0) * 10.0
        for s in range(N_HEAD):
            br[0, s * TP:(s + 1) * TP] = clip8[s]
        for j in range(2):
            br[0, (N_HEAD + j) * TP:(N_HEAD + j + 1) * TP] = clip2[j]
    return br.astype(BF)


def prep_xT(x_b, perm):
    xT = np.zeros((EMBED, TP), dtype=np.float32)
    xT[:, :T] = x_b[perm].T
    return xT.astype(BF)


# ----------------------------------------------------------------------------
# bass kernel emission
# ----------------------------------------------------------------------------
def emit_kernel(tc, ins, out_ap, plans, mask_offs):
    from contextlib import ExitStack
    from concourse import mybir

    nc = tc.nc
    f32 = mybir.dt.float32
    bf16 = mybir.dt.bfloat16
    AF = mybir.ActivationFunctionType
    MULT = mybir.AluOpType.mult

    with ExitStack() as ctx:
        P = ctx.enter_context(tc.tile_pool(name="persist", bufs=1))

        # ---------------- persistent tiles ----------------
        xT = [P.tile([128, TP], bf16, name=f"x{k}", tag=f"x{k}") for k in range(4)]
        qT = [P.tile([128, TP], bf16, name=f"qT{m}", tag=f"qT{m}") for m in range(4)]
        kT = [P.tile([128, TP], bf16, name=f"kT{m}", tag=f"kT{m}") for m in range(4)]
        qml = P.tile([128, TP], bf16, name="qml", tag="qml")
        kml = P.tile([128, TP], bf16, name="kml", tag="kml")
        vext = [P.tile([128, N_HEAD * 65], bf16, name=f"vext{t}", tag=f"vext{t}")
                for t in range(NJB)]
        ytmp = P.tile([65, NSM * TP], f32, name="ytmp", tag="ytmp")
        yTn = [P.tile([128, TP], bf16, name=f"yTn{p}", tag=f"yTn{p}") for p in range(4)]
        tmpml = P.tile([128, TP], bf16, name="tmpml", tag="tmpml")

        def loadw(name, ap, kchunks, ncols, eng, dt=bf16):
            tiles = []
            for kc in range(kchunks):
                t = P.tile([128, ncols], dt, name=f"{name}{kc}", tag=f"{name}{kc}")
                eng.dma_start(t[:], ap[kc * 128:(kc + 1) * 128, :])
                tiles.append(t)
            return tiles

        def loads(name, shape, eng, dt=f32):
            t = P.tile(list(shape), dt, name=name, tag=name)
            eng.dma_start(t[:], ins[name][:, :])
            return t

        # ---------------- input loads ----------------
        # x in per-(kc, ic) chunks, ic-major, so the first projection group
        # can start after ~1/3 of x has landed; wq/wk early on the 2nd queue.
        for kc in range(4):
            nc.sync.dma_start(xT[kc][:, 0:384], ins["xT"][kc * 128:(kc + 1) * 128, 0:384])
        wq_sb = loadw("wq", ins["wq"], 4, 512, nc.scalar)
        wk_sb = loadw("wk", ins["wk"], 4, 512, nc.scalar)
        for ic in (1, 2):
            for kc in range(4):
                nc.sync.dma_start(xT[kc][:, ic * 384:(ic + 1) * 384],
                                  ins["xT"][kc * 128:(kc + 1) * 128, ic * 384:(ic + 1) * 384])
        wv_sb = loadw("wv", ins["wv"], 4, 512, nc.sync)
        wqml_sb = loadw("wqml", ins["wqml"], 4, 128, nc.scalar)
        wkml_sb = loadw("wkml", ins["wkml"], 4, 128, nc.scalar)
        bq_sb = loads("bq", (128, 4), nc.scalar)
        bk_sb = loads("bk", (128, 4), nc.scalar)
        bqml_sb = loads("bqml", (128, 1), nc.scalar)
        bkml_sb = loads("bkml", (128, 1), nc.scalar)
        mixbc_sb = loads("mixbc", (128, NSM), nc.scalar)
        indrow_sb = loads("indrow", (1, TP), nc.scalar, bf16)
        biasrow_sb = loads("biasrow", (1, NSM * TP), nc.sync, bf16)
        onesh_sb = loads("onesh", (128, N_HEAD), nc.sync, bf16)
        maskw = ins["masks"].shape[1]
        maskcat_sb = loads("masks", (128, maskw), nc.sync, bf16)

        mask_eng = [0]

        def proj_group(wtiles, btile, m, dst):
            """One output m-chunk of a projection: psum group + wide evac."""
            ps = _SP3[0].tile([128, 1536], f32, name="pp", tag="sp")
            for ic in range(NIC):
                for kc in range(4):
                    nc.tensor.matmul(
                        ps[:, ic * 512:ic * 512 + 384],
                        lhsT=wtiles[kc][:, m * 128:(m + 1) * 128],
                        rhs=xT[kc][:, ic * 384:(ic + 1) * 384],
                        start=(kc == 0), stop=(kc == 3))
            nc.vector.tensor_scalar_add(
                dst[:].rearrange("p (g w) -> p g w", w=384),
                ps[:].rearrange("p (g c) -> p g c", c=512)[:, :, 0:384],
                btile[:, m:m + 1])

        def att_ic(s, ic):
            _, kindname, src, hv = SM_INFO[s]
            if src == "main":
                qt, kt, off = qT[s // 2], kT[s // 2], (s % 2) * 64
            else:
                qt, kt, off = qml, kml, (s - N_HEAD) * 64
            i0, W = ic * 384, AW[ic]
            units = plans[kindname][ic]
            n_av = sum(len(u["blocks"]) for u in units)
            Y = _YP[0].tile([128, 512], f32, name="y", tag="y")
            avi = 0
            for u in units:
                rows, w, nfit, blocks = u["rows"], u["w"], u["nfit"], u["blocks"]
                nb = len(blocks)
                ps = _SP3[0].tile([128, 1536], f32, name="sp", tag="sp")
                poffs = [(k // nfit) * 512 + (k % nfit) * w for k in range(nb)]
                for k, b in enumerate(blocks):
                    jb, a0 = b["jb"], b["a0"]
                    o = poffs[k]
                    first = (k % nfit == 0)
                    last = (k % nfit == nfit - 1) or (k == nb - 1)
                    nc.tensor.matmul(
                        ps[0:rows, o:o + w],
                        lhsT=kt[off:off + 64, jb * 128:jb * 128 + rows],
                        rhs=qt[off:off + 64, i0 + a0:i0 + a0 + w],
                        start=first, stop=last and (jb != 0))
                    if jb == 0:
                        nc.tensor.matmul(
                            ps[0:rows, o:o + w],
                            lhsT=indrow_sb[0:1, 0:rows],
                            rhs=biasrow_sb[0:1, s * TP + i0 + a0:s * TP + i0 + a0 + w],
                            start=False, stop=last)
                pt = _PTP[0].tile([128, 1536], bf16, name="pt", tag="pt")
                if nfit == 1:
                    pin = ps[0:rows, :].rearrange("p (g c) -> p g c", c=512)[:, 0:nb, 0:w]
                    pout = pt[0:rows, 0:nb * w].rearrange("p (g c) -> p g c", c=w)
                else:
                    pin = ps[0:rows, 0:nb * w]
                    pout = pt[0:rows, 0:nb * w]
                nc.scalar.activation(pout, pin, AF.Exp)
                for k, b in enumerate(blocks):
                    if b["mid"] is not None:
                        mo, mw = mask_offs[b["mid"]]
                        eng = nc.vector if (mask_eng[0] % 2 == 0) else nc.gpsimd
                        mask_eng[0] += 1
                        eng.tensor_mul(pt[0:rows, k * w:(k + 1) * w],
                                       pt[0:rows, k * w:(k + 1) * w],
                                       maskcat_sb[0:rows, mo:mo + mw])
                for k, b in enumerate(blocks):
                    a0 = b["a0"]
                    nc.tensor.matmul(
                        Y[0:65, a0:a0 + w],
                        lhsT=vext[b["jb"]][0:rows, hv * 65:hv * 65 + 65],
                        rhs=pt[0:rows, k * w:(k + 1) * w],
                        start=(avi == 0), stop=(avi == n_av - 1))
                    avi += 1
            nc.vector.tensor_copy(ytmp[0:65, s * TP + i0:s * TP + i0 + W], Y[0:65, 0:W])

        def norm(s):
            zr = _ZRP[0].tile([1, TP], bf16, name="zr", tag="zr")
            with nc.allow_low_precision(reason="bf16 softmax normalizer, validated"):
                nc.vector.reciprocal(zr[0:1, 0:T], ytmp[64:65, s * TP:s * TP + T])
            rb = _RBP[0].tile([128, TP], bf16, name="rb", tag="rb")
            nc.gpsimd.partition_broadcast(rb[0:128, 0:T], zr[0:1, 0:T], channels=128)
            if s < N_HEAD:
                dst = yTn[s // 2][(s % 2) * 64:(s % 2) * 64 + 64, 0:T]
            else:
                dst = tmpml[(s - N_HEAD) * 64:(s - N_HEAD) * 64 + 64, 0:T]
            nc.vector.scalar_tensor_tensor(
                dst, ytmp[0:64, s * TP:s * TP + T], mixbc_sb[0:64, s:s + 1],
                rb[0:64, 0:T], MULT, MULT)

        # ---------------- phase A: q1/k1/v projections ----------------
        with tc.tile_pool(name="pps", bufs=2, space="PSUM") as pps, \
             tc.tile_pool(name="vps", bufs=2, space="PSUM") as vps:
            _SP3 = [pps]
            proj_group(wq_sb, bq_sb, 1, qT[1])
            proj_group(wk_sb, bk_sb, 1, kT[1])
            for tt in range(NJB):
                ps = vps.tile([128, 512], f32, name="vp", tag="vp")
                for kc in range(4):
                    nc.tensor.matmul(
                        ps[:],
                        lhsT=xT[kc][:, tt * 128:(tt + 1) * 128],
                        rhs=wv_sb[kc][:],
                        start=(kc == 0), stop=(kc == 3))
                vx = vext[tt][:].rearrange("p (h e) -> p h e", e=65)
                nc.scalar.activation(vx[:, :, 0:64],
                                     ps[:].rearrange("p (h d) -> p h d", d=64),
                                     AF.Copy)
                nc.sync.dma_start(vx[:, :, 64:65], ins["onesh"][:, :, None])

        wp_sb = loadw("wp", ins["wp"], 4, 512, nc.sync)

        # ---------------- phase B: interleaved projections + attention ----
        with tc.tile_pool(name="sp3", bufs=2, space="PSUM") as sp3, \
             tc.tile_pool(name="yp", bufs=2, space="PSUM") as yp, \
             tc.tile_pool(name="ptp", bufs=4) as ptp, \
             tc.tile_pool(name="zrp", bufs=2) as zrp, \
             tc.tile_pool(name="rbp", bufs=2) as rbp:
            _SP3[0] = sp3
            _YP = [yp]
            _PTP = [ptp]
            _ZRP = [zrp]
            _RBP = [rbp]

            # zero the padding columns of the normalized tiles once
            for tile_ in yTn + [tmpml]:
                nc.gpsimd.memset(tile_[:, T:TP], 0.0)

            def A(s):
                for ic in range(NIC):
                    att_ic(s, ic)
                norm(s)

            proj_work = [
                (wq_sb, bq_sb, 2, qT[2]), (wk_sb, bk_sb, 2, kT[2]),
                (wq_sb, bq_sb, 3, qT[3]), (wk_sb, bk_sb, 3, kT[3]),
                (wq_sb, bq_sb, 0, qT[0]), (wk_sb, bk_sb, 0, kT[0]),
                (wqml_sb, bqml_sb, 0, qml), (wkml_sb, bkml_sb, 0, kml),
            ]
            att_work = SM_ORDER
            sched = ["P", "A", "P", "A", "P", "A", "P", "A",
                     "P", "A", "P", "A", "P", "A", "P", "A", "A", "A"]
            pi = ai = 0
            for kind_w in sched:
                if kind_w == "P":
                    proj_group(*proj_work[pi])
                    pi += 1
                else:
                    A(att_work[ai])
                    ai += 1
            nc.vector.tensor_add(yTn[1][:, 0:T], yTn[1][:, 0:T], tmpml[:, 0:T])

        # ---------------- phase C: output projection ----------------
        with tc.tile_pool(name="ops", bufs=2, space="PSUM") as ops, \
             tc.tile_pool(name="ostage", bufs=3) as ostage:
            for m in range(NJB):
                po = ops.tile([128, 512], f32, name="po", tag="po")
                for i, p in enumerate((0, 2, 3, 1)):
                    nc.tensor.matmul(
                        po[:],
                        lhsT=yTn[p][:, m * 128:(m + 1) * 128],
                        rhs=wp_sb[p][:],
                        start=(i == 0), stop=(i == 3))
                ost = ostage.tile([128, 512], f32, name="ost", tag="ost")
                if m % 2 == 0:
                    nc.scalar.activation(ost[:], po[:], AF.Copy)
                    nc.sync.dma_start(out_ap[m * 128:(m + 1) * 128, :], ost[:])
                else:
                    nc.vector.tensor_copy(ost[:], po[:])
                    nc.scalar.dma_start(out_ap[m * 128:(m + 1) * 128, :], ost[:])


# ----------------------------------------------------------------------------
# module build + run
# ----------------------------------------------------------------------------
_CACHE = {}


def _get_module():
    if "nc" in _CACHE:
        return _CACHE["nc"], _CACHE["maskcat"]
    import concourse.tile as tile
    from concourse import bacc, mybir

    plans, maskcat, mask_offs = build_units()

    nc = bacc.Bacc("TRN2", target_bir_lowering=False, debug=False,
                   enable_asserts=False, num_devices=NCORES)
    f32 = mybir.dt.float32
    bf16 = mybir.dt.bfloat16

    def din(name, shape, dt=f32):
        return nc.dram_tensor(name, list(shape), dt, kind="ExternalInput").ap()

    ins = dict(
        xT=din("xT", (EMBED, TP), bf16),
        wq=din("wq", (EMBED, EMBED), bf16), wk=din("wk", (EMBED, EMBED), bf16),
        wv=din("wv", (EMBED, EMBED), bf16), wp=din("wp", (EMBED, EMBED), bf16),
        wqml=din("wqml", (EMBED, 128), bf16), wkml=din("wkml", (EMBED, 128), bf16),
        bq=din("bq", (128, 4)), bk=din("bk", (128, 4)),
        bqml=din("bqml", (128, 1)), bkml=din("bkml", (128, 1)),
        mixbc=din("mixbc", (128, NSM)),
        indrow=din("indrow", (1, TP), bf16),
        biasrow=din("biasrow", (1, NSM * TP), bf16),
        onesh=din("onesh", (128, N_HEAD), bf16),
        masks=din("masks", (128, maskcat.shape[1]), bf16),
    )
    out_ap = nc.dram_tensor("out_p", [TP, EMBED], f32, kind="ExternalOutput").ap()

    with tile.TileContext(nc) as tc:
        emit_kernel(tc, ins, out_ap, plans, mask_offs)
    nc.compile()

    _CACHE.update(nc=nc, maskcat=maskcat.astype(BF))
    return nc, _CACHE["maskcat"]


def build_in_maps(inputs):
    nc, maskcat = _get_module()
    x = inputs["x"].astype(np.float32)
    cond = np.asarray(inputs["cond_mask"]).astype(np.int32)
    B = x.shape[0]
    assert B == NCORES, f"expected B={NCORES}, got {B}"

    perm, _ = build_perm()
    shared = prep_shared(inputs)
    shared["masks"] = maskcat
    br_cache = {}
    in_maps = []
    for b in range(B):
        ci = dict(shared)
        ci["xT"] = prep_xT(x[b], perm)
        cb = int(cond[b])
        if cb not in br_cache:
            br_cache[cb] = prep_biasrow(inputs, cb)
        ci["biasrow"] = br_cache[cb]
        in_maps.append(ci)
    return nc, in_maps


def kernel(**inputs):
    from concourse import bass_utils

    inputs = {k: np.asarray(v) for k, v in inputs.items()}
    nc, in_maps = build_in_maps(inputs)
    res = bass_utils.run_bass_kernel_spmd(nc, in_maps, core_ids=list(range(NCORES)))
    _CACHE["last_results"] = res

    _, inv = build_perm()
    shift = host_const_shift(inputs)
    B = inputs["x"].shape[0]
    out = np.empty((B, T, EMBED), dtype=np.float32)
    for b in range(B):
        out[b] = res.results[b]["out_p"][:T][inv] + shift
    return out


# revision 10
# speedup vs baseline: 1.3781x; 1.1816x over previous
"""Trainium2 Bass kernel for nn_CausalCrossConditionalSelfAttention.

Strategy (8 NeuronCores, data-parallel over batch B=8, one element/core):
  - Host permutes tokens to interleaved temporal order => causal mask becomes
    lower-triangular, local mask a narrow band (+2 prefix cols).
  - All matmuls in bf16 (1 cyc/row at any N in the TRN2 cost model), psum fp32.
  - Scores computed transposed S^T[key, query] in column-cropped blocks;
    band blocks cropped to their true content span; fully-masked blocks
    skipped; partial blocks multiplied by host-precomputed 0/1 bf16 masks.
  - Conditional CLIP-token bias added via a rank-1 accumulate matmul
    (indicator-row x bias-row) on jb==0 score blocks only.
  - exp() batched: several score blocks share one psum tile and one wide
    Act instruction (Act is the attention-phase bottleneck engine).
  - P@[V|1] gives unnormalized y plus softmax denominator Z as psum row 64;
    both evacuated per (softmax, query-chunk) to an fp32 staging tile; Z
    reciprocal on DVE -> gpsimd partition_broadcast -> fused
    (y * mix) * (1/Z) normalization via scalar_tensor_tensor into yTn bf16.
  - Projections interleaved with attention so PE stays dense (full clock);
    q/k projections for the first softmaxes run first.
  - b_value/b_proj folded into a constant host-side output shift.

Self-contained: only needs numpy + ml_dtypes + the installed concourse stack.
"""

import sys

if "/opt/trn_rl_repo" not in sys.path:
    sys.path.insert(0, "/opt/trn_rl_repo")

import numpy as np
import ml_dtypes

# ----------------------------------------------------------------------------
# problem constants (hardcoded per spec)
# ----------------------------------------------------------------------------
BLOCK = 512
RECEP = 4
N_HEAD = 8
EMBED = 512
HS = 64
T = 2 * BLOCK + 2          # 1026
TP = 1152                  # 9 * 128
NJB = TP // 128
NIC = 3
AW = (384, 384, 258)       # per-ic live query widths (ic2 ends at T=1026)
NSM = 10
NCORES = 8
BF = ml_dtypes.bfloat16

# softmax id -> (mask kind, q/k source, v head)
SM_INFO = [
    (0, "loc", "main", 0), (1, "loc", "main", 1),
    (2, "seq", "main", 2), (3, "seq", "main", 3),
    (4, "seq", "main", 4), (5, "seq", "main", 5),
    (6, "seq", "main", 6), (7, "seq", "main", 7),
    (8, "loc", "ml", 2), (9, "loc", "ml", 3),
]
# emission order: heavy causal heads first, cheap band heads last
SM_ORDER = [2, 3, 4, 5, 6, 7, 0, 8, 1, 9]


# ----------------------------------------------------------------------------
# host-side plan construction
# ----------------------------------------------------------------------------
def build_perm():
    perm = np.zeros(T, dtype=np.int64)
    perm[0], perm[1] = 0, 1
    b = np.arange(BLOCK)
    perm[2 + 2 * b] = 2 + b
    perm[3 + 2 * b] = 2 + BLOCK + b
    inv = np.argsort(perm)
    return perm, inv


def build_masks_orig():
    to = np.concatenate([np.zeros(2), np.arange(BLOCK) * 2 + 1, np.arange(BLOCK) * 2 + 2])
    seq = to[None, :] <= to[:, None]
    qo = np.concatenate([np.arange(BLOCK) * 2 + 1 - 2 * RECEP + 1] * 2)
    ko = np.concatenate([np.arange(BLOCK) * 2 + 1] * 2)
    de = ko[None, :] < qo[:, None]
    loc = seq.copy()
    loc[2:, 2:] = loc[2:, 2:] & (~de)
    return seq, loc


def build_units():
    """Per (kind, ic): list of units.

    unit = dict(rows, w, nfit, blocks=[dict(jb, a0, mid)]); blocks in a unit
    share (rows, w); psum layout: single-bank packed (nfit>1, chunk k at
    col k*w) or one bank per block (nfit==1, chunk k at col k*512).
    Unit 0 contains jb==0 (AV start flag / CLIP bias matmul target).
    """
    perm, _ = build_perm()
    seq, loc = build_masks_orig()
    mask_tiles, tile_index = [], {}

    def tile_id(slab):
        key = (slab.shape[1], slab.tobytes())
        if key not in tile_index:
            tile_index[key] = len(mask_tiles)
            mask_tiles.append(slab.astype(np.float32))
        return tile_index[key]

    plans = {}
    for kind, M0 in (("seq", seq), ("loc", loc)):
        Mp = np.zeros((TP, TP), dtype=bool)
        Mp[:T, :T] = M0[perm][:, perm]
        icunits = []
        for ic in range(NIC):
            i0, W = ic * 384, AW[ic]
            blocks = []
            for jb in range(NJB):
                sub = Mp[i0:i0 + W, jb * 128:(jb + 1) * 128].T.copy()  # [128 keys, W]
                if not sub.any():
                    continue
                nzr = np.flatnonzero(sub.any(axis=1))
                rows = 2 if nzr.max() <= 1 else 128
                nzc = np.flatnonzero(sub.any(axis=0))
                a0 = int(nzc.min()) & ~1
                a1 = min(W, (int(nzc.max()) + 2) & ~1)
                blocks.append((jb, rows, a0, a1, sub))
            # classes by (rows, wide?); unify width within class
            cls = {}
            for b in blocks:
                jb, rows, a0, a1, sub = b
                cls.setdefault((rows, (a1 - a0) >= 256), []).append(b)
            units = []
            for (rows, wide), bl in sorted(cls.items(), key=lambda kv: min(b[0] for b in kv[1])):
                w = W if wide else max(b[3] - b[2] for b in bl)
                nfit = (512 // w) if w < 256 else 1
                cap = nfit if nfit > 1 else 3
                cur = []
                for jb, brows, a0, a1, sub in bl:
                    a0 = max(0, min(a0, W - w)) & ~1
                    slab = sub[:, a0:a0 + w]
                    mid = None if slab[0:rows].all() else tile_id(slab)
                    cur.append(dict(jb=jb, a0=a0, mid=mid))
                    if len(cur) == cap:
                        units.append(dict(rows=rows, w=w, nfit=nfit, blocks=cur))
                        cur = []
                if cur:
                    units.append(dict(rows=rows, w=w, nfit=nfit, blocks=cur))
            units.sort(key=lambda u: min(b["jb"] for b in u["blocks"]))
            assert units[0]["blocks"][0]["jb"] == 0
            icunits.append(units)
        plans[kind] = icunits

    offs, cat, o = [], [], 0
    for t in mask_tiles:
        offs.append((o, t.shape[1]))
        cat.append(t)
        o += t.shape[1]
    maskcat = np.concatenate(cat, axis=1) if cat else np.zeros((128, 2), np.float32)
    return plans, maskcat, offs


def host_const_shift(w):
    bv = w["b_value"].astype(np.float64)
    wg = w["w_mix"].astype(np.float64)[:, 0, 0, 0]
    wl = w["w_mix"].astype(np.float64)[:, 1, 0, 0]
    scale_h = np.ones(N_HEAD)
    scale_h[2] = wg[0] + wl[0]
    scale_h[3] = wg[1] + wl[1]
    yshift = (bv.reshape(N_HEAD, HS) * scale_h[:, None]).reshape(-1)
    return (yshift @ w["w_proj"].astype(np.float64).T
            + w["b_proj"].astype(np.float64)).astype(np.float32)


def prep_shared(w):
    """Shared (weight) tensors, bf16 where they feed matmuls."""
    f = np.float32
    scale = np.float32(1.0 / np.sqrt(HS))
    out = {}
    out["wq"] = np.ascontiguousarray(w["w_query"].astype(f).T * scale).astype(BF)
    out["wk"] = np.ascontiguousarray(w["w_key"].astype(f).T).astype(BF)
    out["wv"] = np.ascontiguousarray(w["w_value"].astype(f).T).astype(BF)
    out["wp"] = np.ascontiguousarray(w["w_proj"].astype(f).T).astype(BF)
    out["wqml"] = np.ascontiguousarray(w["w_query_ml"].astype(f).T * scale).astype(BF)
    out["wkml"] = np.ascontiguousarray(w["w_key_ml"].astype(f).T).astype(BF)
    # merged per-partition constants: [bq(4) | bk(4) | bqml | bkml | mixbc(10)]
    consts = np.zeros((128, 20), dtype=f)
    consts[:, 0:4] = (w["b_query"].astype(f) * scale).reshape(4, 128).T
    consts[:, 4:8] = w["b_key"].astype(f).reshape(4, 128).T
    consts[:, 8] = w["b_query_ml"].astype(f) * scale
    consts[:, 9] = w["b_key_ml"].astype(f)
    wg = w["w_mix"].astype(f)[:, 0, 0, 0]
    wl = w["w_mix"].astype(f)[:, 1, 0, 0]
    mix = np.ones(NSM, dtype=f)
    mix[2], mix[3] = wg[0], wg[1]
    mix[8], mix[9] = wl[0], wl[1]
    consts[:, 10:20] = mix[None, :]
    out["consts"] = consts

    ind = np.zeros((1, TP), dtype=f)
    ind[0, 1] = 1.0
    out["indrow"] = ind.astype(BF)
    return out


def prep_biasrow(w, cond_b):
    f = np.float32
    br = np.zeros((1, NSM * TP), dtype=f)
    if cond_b > 0:
        clip8 = np.maximum(w["att_bias_clip"].astype(f)[0, :, 0], 0.0) * 10.0
        clip2 = np.maximum(w["att_bias_clip_ml"].astype(f)[0, :, 0], 0.0) * 10.0
        for s in range(N_HEAD):
            br[0, s * TP:(s + 1) * TP] = clip8[s]
        for j in range(2):
            br[0, (N_HEAD + j) * TP:(N_HEAD + j + 1) * TP] = clip2[j]
    return br.astype(BF)


def prep_xT(x_b, perm):
    xT = np.zeros((EMBED, TP), dtype=np.float32)
    xT[:, :T] = x_b[perm].T
    return xT.astype(BF)


# ----------------------------------------------------------------------------
# bass kernel emission
# ----------------------------------------------------------------------------
def emit_kernel(tc, ins, out_ap, plans, mask_offs):
    from contextlib import ExitStack
    from concourse import mybir

    nc = tc.nc
    f32 = mybir.dt.float32
    bf16 = mybir.dt.bfloat16
    AF = mybir.ActivationFunctionType
    MULT = mybir.AluOpType.mult

    with ExitStack() as ctx:
        P = ctx.enter_context(tc.tile_pool(name="persist", bufs=1))

        # ---------------- persistent tiles ----------------
        # x and weights live in single wide tiles (kc chunks along the free
        # dim) so each loads with one or two big DMAs — per-DMA queue
        # turnaround (~1.3us) dominates many-small-transfer schedules.
        xTb = P.tile([128, 4 * TP], bf16, name="xTb", tag="xTb")
        qT = [P.tile([128, TP], bf16, name=f"qT{m}", tag=f"qT{m}") for m in range(4)]
        kT = [P.tile([128, TP], bf16, name=f"kT{m}", tag=f"kT{m}") for m in range(4)]
        qml = P.tile([128, TP], bf16, name="qml", tag="qml")
        kml = P.tile([128, TP], bf16, name="kml", tag="kml")
        vext = [P.tile([128, N_HEAD * 65], bf16, name=f"vext{t}", tag=f"vext{t}")
                for t in range(NJB)]
        ytmp = P.tile([65, NSM * TP], f32, name="ytmp", tag="ytmp")
        yTn = [P.tile([128, TP], bf16, name=f"yTn{p}", tag=f"yTn{p}") for p in range(4)]
        tmpml = P.tile([128, TP], bf16, name="tmpml", tag="tmpml")

        def xs(kc, c0, c1):
            return xTb[:, kc * TP + c0:kc * TP + c1]

        def loadw(name, nkc, ncols, eng):
            """Whole [512, ncols] weight as one [128, 4*ncols] tile, one DMA."""
            t = P.tile([128, nkc * ncols], bf16, name=name, tag=name)
            eng.dma_start(t[:].rearrange("p (g c) -> p g c", c=ncols),
                          ins[name].rearrange("(g p) c -> p g c", p=128))
            return t

        # ---------------- input loads ----------------
        # first projection group needs x(ic0) + wq only; interleave queues.
        wqb = loadw("wq", 4, 512, nc.sync)
        wkb = loadw("wk", 4, 512, nc.scalar)
        for ic in range(NIC):
            c0, c1 = ic * 384, (ic + 1) * 384
            nc.sync.dma_start(
                xTb[:].rearrange("p (g c) -> p g c", c=TP)[:, 0:2, c0:c1],
                ins["xT"].rearrange("(g p) c -> p g c", p=128)[:, 0:2, c0:c1])
            nc.scalar.dma_start(
                xTb[:].rearrange("p (g c) -> p g c", c=TP)[:, 2:4, c0:c1],
                ins["xT"].rearrange("(g p) c -> p g c", p=128)[:, 2:4, c0:c1])
        wvb = loadw("wv", 4, 512, nc.sync)
        wqmlb = loadw("wqml", 4, 128, nc.scalar)
        wkmlb = loadw("wkml", 4, 128, nc.scalar)
        consts_sb = P.tile([128, 20], f32, name="consts", tag="consts")
        nc.scalar.dma_start(consts_sb[:], ins["consts"][:, :])
        indrow_sb = P.tile([1, TP], bf16, name="indrow", tag="indrow")
        nc.scalar.dma_start(indrow_sb[:], ins["indrow"][:, :])
        biasrow_sb = P.tile([1, NSM * TP], bf16, name="biasrow", tag="biasrow")
        nc.sync.dma_start(biasrow_sb[:], ins["biasrow"][:, :])
        maskw = ins["masks"].shape[1]
        maskcat_sb = P.tile([128, maskw], bf16, name="masks", tag="masks")
        nc.sync.dma_start(maskcat_sb[:], ins["masks"][:, :])

        def proj_group(wtile, bcol, m, dst):
            """One output m-chunk of a projection: psum group + wide evac."""
            ps = _SP3[0].tile([128, 1536], f32, name="pp", tag="sp")
            for ic in range(NIC):
                for kc in range(4):
                    nc.tensor.matmul(
                        ps[:, ic * 512:ic * 512 + 384],
                        lhsT=wtile[:, kc * (wtile.shape[1] // 4) + m * 128:
                                   kc * (wtile.shape[1] // 4) + (m + 1) * 128],
                        rhs=xs(kc, ic * 384, (ic + 1) * 384),
                        start=(kc == 0), stop=(kc == 3))
            nc.vector.tensor_scalar_add(
                dst[:].rearrange("p (g w) -> p g w", w=384),
                ps[:].rearrange("p (g c) -> p g c", c=512)[:, :, 0:384],
                consts_sb[:, bcol:bcol + 1])

        def att_ic(s, ic):
            _, kindname, src, hv = SM_INFO[s]
            if src == "main":
                qt, kt, off = qT[s // 2], kT[s // 2], (s % 2) * 64
            else:
                qt, kt, off = qml, kml, (s - N_HEAD) * 64
            i0, W = ic * 384, AW[ic]
            units = plans[kindname][ic]
            n_av = sum(len(u["blocks"]) for u in units)
            Y = _YP[0].tile([128, 512], f32, name="y", tag="y")
            avi = 0
            for u in units:
                rows, w, nfit, blocks = u["rows"], u["w"], u["nfit"], u["blocks"]
                nb = len(blocks)
                ps = _SP3[0].tile([128, 1536], f32, name="sp", tag="sp")
                poffs = [(k // nfit) * 512 + (k % nfit) * w for k in range(nb)]
                for k, b in enumerate(blocks):
                    jb, a0 = b["jb"], b["a0"]
                    o = poffs[k]
                    first = (k % nfit == 0)
                    last = (k % nfit == nfit - 1) or (k == nb - 1)
                    nc.tensor.matmul(
                        ps[0:rows, o:o + w],
                        lhsT=kt[off:off + 64, jb * 128:jb * 128 + rows],
                        rhs=qt[off:off + 64, i0 + a0:i0 + a0 + w],
                        start=first, stop=last and (jb != 0))
                    if jb == 0:
                        nc.tensor.matmul(
                            ps[0:rows, o:o + w],
                            lhsT=indrow_sb[0:1, 0:rows],
                            rhs=biasrow_sb[0:1, s * TP + i0 + a0:s * TP + i0 + a0 + w],
                            start=False, stop=last)
                pt = _PTP[0].tile([128, 1536], bf16, name="pt", tag="pt")
                if nfit == 1:
                    pin = ps[0:rows, :].rearrange("p (g c) -> p g c", c=512)[:, 0:nb, 0:w]
                    pout = pt[0:rows, 0:nb * w].rearrange("p (g c) -> p g c", c=w)
                else:
                    pin = ps[0:rows, 0:nb * w]
                    pout = pt[0:rows, 0:nb * w]
                nc.scalar.activation(pout, pin, AF.Exp)
                for k, b in enumerate(blocks):
                    if b["mid"] is not None:
                        mo, mw = mask_offs[b["mid"]]
                        nc.vector.tensor_mul(pt[0:rows, k * w:(k + 1) * w],
                                             pt[0:rows, k * w:(k + 1) * w],
                                             maskcat_sb[0:rows, mo:mo + mw])
                for k, b in enumerate(blocks):
                    a0 = b["a0"]
                    nc.tensor.matmul(
                        Y[0:65, a0:a0 + w],
                        lhsT=vext[b["jb"]][0:rows, hv * 65:hv * 65 + 65],
                        rhs=pt[0:rows, k * w:(k + 1) * w],
                        start=(avi == 0), stop=(avi == n_av - 1))
                    avi += 1
            nc.vector.tensor_copy(ytmp[0:65, s * TP + i0:s * TP + i0 + W], Y[0:65, 0:W])

        def norm(s):
            zr = _ZRP[0].tile([1, TP], bf16, name="zr", tag="zr")
            with nc.allow_low_precision(reason="bf16 softmax normalizer, validated"):
                nc.vector.reciprocal(zr[0:1, 0:T], ytmp[64:65, s * TP:s * TP + T])
            rb = _RBP[0].tile([128, TP], bf16, name="rb", tag="rb")
            nc.gpsimd.partition_broadcast(rb[0:128, 0:T], zr[0:1, 0:T], channels=128)
            if s < N_HEAD:
                dst = yTn[s // 2][(s % 2) * 64:(s % 2) * 64 + 64, 0:T]
            else:
                dst = tmpml[(s - N_HEAD) * 64:(s - N_HEAD) * 64 + 64, 0:T]
            nc.vector.scalar_tensor_tensor(
                dst, ytmp[0:64, s * TP:s * TP + T], consts_sb[0:64, 10 + s:11 + s],
                rb[0:64, 0:T], MULT, MULT)

        # ---------------- phase A: q1/k1/v projections ----------------
        with tc.tile_pool(name="pps", bufs=2, space="PSUM") as pps, \
             tc.tile_pool(name="vps", bufs=2, space="PSUM") as vps:
            _SP3 = [pps]
            proj_group(wqb, 0 + 1, 1, qT[1])
            proj_group(wkb, 4 + 1, 1, kT[1])
            for tt in range(NJB):
                ps = vps.tile([128, 512], f32, name="vp", tag="vp")
                for kc in range(4):
                    nc.tensor.matmul(
                        ps[:],
                        lhsT=xs(kc, tt * 128, (tt + 1) * 128),
                        rhs=wvb[:, kc * 512:(kc + 1) * 512],
                        start=(kc == 0), stop=(kc == 3))
                vx = vext[tt][:].rearrange("p (h e) -> p h e", e=65)
                nc.scalar.activation(vx[:, :, 0:64],
                                     ps[:].rearrange("p (h d) -> p h d", d=64),
                                     AF.Copy)
                nc.gpsimd.memset(vx[:, :, 64:65], 1.0)

        wpb = loadw("wp", 4, 512, nc.sync)

        # ---------------- phase B: interleaved projections + attention ----
        with tc.tile_pool(name="sp3", bufs=2, space="PSUM") as sp3, \
             tc.tile_pool(name="yp", bufs=2, space="PSUM") as yp, \
             tc.tile_pool(name="ptp", bufs=4) as ptp, \
             tc.tile_pool(name="zrp", bufs=2) as zrp, \
             tc.tile_pool(name="rbp", bufs=2) as rbp:
            _SP3[0] = sp3
            _YP = [yp]
            _PTP = [ptp]
            _ZRP = [zrp]
            _RBP = [rbp]

            # zero the padding columns of the normalized tiles once
            for tile_ in yTn + [tmpml]:
                nc.gpsimd.memset(tile_[:, T:TP], 0.0)

            pending = [None]

            def A(s):
                # norm of the previous softmax is emitted behind this one's
                # first chunk so its DVE/Pool latency hides under fresh work
                att_ic(s, 0)
                if pending[0] is not None:
                    norm(pending[0])
                att_ic(s, 1)
                att_ic(s, 2)
                pending[0] = s

            proj_work = [
                (wqb, 0 + 2, 2, qT[2]), (wkb, 4 + 2, 2, kT[2]),
                (wqb, 0 + 3, 3, qT[3]), (wkb, 4 + 3, 3, kT[3]),
                (wqb, 0 + 0, 0, qT[0]), (wkb, 4 + 0, 0, kT[0]),
                (wqmlb, 8, 0, qml), (wkmlb, 9, 0, kml),
            ]
            att_work = SM_ORDER
            sched = ["P", "A", "P", "A", "P", "A", "P", "A",
                     "P", "A", "P", "A", "P", "A", "P", "A", "A", "A"]
            pi = ai = 0
            for kind_w in sched:
                if kind_w == "P":
                    proj_group(*proj_work[pi])
                    pi += 1
                else:
                    A(att_work[ai])
                    ai += 1
            norm(pending[0])
            nc.vector.tensor_add(yTn[1][:, 0:T], yTn[1][:, 0:T], tmpml[:, 0:T])

        # ---------------- phase C: output projection ----------------
        with tc.tile_pool(name="ops", bufs=2, space="PSUM") as ops, \
             tc.tile_pool(name="ostage", bufs=3) as ostage:
            ost = None
            for m in range(NJB):
                po = ops.tile([128, 512], f32, name="po", tag="po")
                for i, p in enumerate((2, 3, 0, 1)):
                    nc.tensor.matmul(
                        po[:],
                        lhsT=yTn[p][:, m * 128:(m + 1) * 128],
                        rhs=wpb[:, p * 512:(p + 1) * 512],
                        start=(i == 0), stop=(i == 3))
                if m % 2 == 0:
                    ost = ostage.tile([128, 1024], f32, name="ost", tag="ost")
                    nc.scalar.activation(ost[:, 0:512], po[:], AF.Copy)
                    if m == NJB - 1:
                        nc.sync.dma_start(out_ap[m * 128:(m + 1) * 128, :],
                                          ost[:, 0:512])
                else:
                    nc.vector.tensor_copy(ost[:, 512:1024], po[:])
                    eng = nc.sync if m % 4 == 1 else nc.scalar
                    eng.dma_start(
                        out_ap[(m - 1) * 128:(m + 1) * 128, :].rearrange(
                            "(g p) c -> p g c", p=128),
                        ost[:].rearrange("p (g c) -> p g c", c=512))


# ----------------------------------------------------------------------------
# module build + run
# ----------------------------------------------------------------------------
_CACHE = {}


def _get_module():
    if "nc" in _CACHE:
        return _CACHE["nc"], _CACHE["maskcat"]
    import concourse.tile as tile
    from concourse import bacc, mybir

    plans, maskcat, mask_offs = build_units()

    nc = bacc.Bacc("TRN2", target_bir_lowering=False, debug=False,
                   enable_asserts=False, num_devices=NCORES)
    f32 = mybir.dt.float32
    bf16 = mybir.dt.bfloat16

    def din(name, shape, dt=f32):
        return nc.dram_tensor(name, list(shape), dt, kind="ExternalInput").ap()

    ins = dict(
        xT=din("xT", (EMBED, TP), bf16),
        wq=din("wq", (EMBED, EMBED), bf16), wk=din("wk", (EMBED, EMBED), bf16),
        wv=din("wv", (EMBED, EMBED), bf16), wp=din("wp", (EMBED, EMBED), bf16),
        wqml=din("wqml", (EMBED, 128), bf16), wkml=din("wkml", (EMBED, 128), bf16),
        consts=din("consts", (128, 20)),
        indrow=din("indrow", (1, TP), bf16),
        biasrow=din("biasrow", (1, NSM * TP), bf16),
        masks=din("masks", (128, maskcat.shape[1]), bf16),
    )
    out_ap = nc.dram_tensor("out_p", [TP, EMBED], f32, kind="ExternalOutput").ap()

    with tile.TileContext(nc) as tc:
        emit_kernel(tc, ins, out_ap, plans, mask_offs)
    nc.compile()

    _CACHE.update(nc=nc, maskcat=maskcat.astype(BF))
    return nc, _CACHE["maskcat"]


def build_in_maps(inputs):
    nc, maskcat = _get_module()
    x = inputs["x"].astype(np.float32)
    cond = np.asarray(inputs["cond_mask"]).astype(np.int32)
    B = x.shape[0]
    assert B == NCORES, f"expected B={NCORES}, got {B}"

    perm, _ = build_perm()
    shared = prep_shared(inputs)
    shared["masks"] = maskcat
    br_cache = {}
    in_maps = []
    for b in range(B):
        ci = dict(shared)
        ci["xT"] = prep_xT(x[b], perm)
        cb = int(cond[b])
        if cb not in br_cache:
            br_cache[cb] = prep_biasrow(inputs, cb)
        ci["biasrow"] = br_cache[cb]
        in_maps.append(ci)
    return nc, in_maps


def kernel(**inputs):
    from concourse import bass_utils

    inputs = {k: np.asarray(v) for k, v in inputs.items()}
    nc, in_maps = build_in_maps(inputs)
    res = bass_utils.run_bass_kernel_spmd(nc, in_maps, core_ids=list(range(NCORES)))
    _CACHE["last_results"] = res

    _, inv = build_perm()
    shift = host_const_shift(inputs)
    B = inputs["x"].shape[0]
    out = np.empty((B, T, EMBED), dtype=np.float32)
    for b in range(B):
        out[b] = res.results[b]["out_p"][:T][inv] + shift
    return out
